# revision 3
# baseline (speedup 1.0000x reference)
"""Trainium2 Bass kernel: 4096x4096 single-channel 3x3 VALID conv + bias.

Sharding: 8-way row-parallel. Core i computes output rows [512*i, 512*i+512)
(core 7: 510 valid rows). Halo handled host-side: each core's input shard is
[514, 4096] (512 rows + 2 halo rows; core 7 zero-padded).

v3: fp16 everywhere on device (tolerance 2e-2; fp16 end-to-end measures
~6.6e-4), halving HBM traffic. Host converts fp32<->fp16 around the run.

Per core: 4 main stripes of 126 output rows + an 8-row stub. A main stripe's
128 input rows sit on SBUF partitions; per 512-wide PSUM bank, 3 matmuls (one
per kernel column dj, rhs shifted by dj along the free dim) against 128x126
fp16 band matrices accumulate all 9 taps into fp32 PSUM. The stub instead
uses a [30, 4094] tile holding the last 10 input rows pre-shifted by dj
(3 extra tiny DRAM loads) and a [30, 8] band, so all 9 taps cost ONE matmul
per bank. Stripe 0 is loaded in 4 column chunks so the PE can start after
~260KB. ScalarE (even banks) / VectorE (odd banks) evacuate PSUM->SBUF
fusing +bias and the fp32->fp16 cast; scalar's HWDGE ring stores each half
stripe; sync's HWDGE ring streams the input.

Sync notes (hard-won):
- This walrus build allows at most ONE sem wait and ONE sem update per
  instruction; extra waits are standalone engine.wait_ge() instructions.
- An HWDGE dma_start on a compute engine's queue does NOT wait for prior
  compute writes to land; the DMA must be gated by a sem incremented by the
  last producing instruction, or it reads stale SBUF.
"""

import numpy as np

import concourse.bass as bass
import concourse.mybir as mybir
from concourse.bass_utils import run_bass_kernel_spmd

H = W = 4096
KH = KW = 3
OH = OW = H - KH + 1  # 4094
NCORES = 8
CROWS = 512            # output rows per core (core 7: 510 valid)
IN_ROWS = CROWS + KH - 1  # 514 input rows per core shard
STRIPE = 126           # output rows per full main stripe
N_MAIN = 4             # main stripes; stub covers rows [504, 512)
STUB_R0 = N_MAIN * STRIPE   # 504
STUB_ROWS = CROWS - STUB_R0  # 8
NBANKS = 8             # PSUM banks; bank b covers output cols [512b, 512b+Nb)
HALF_COL = 2048        # output halves: [0, 2048) and [2048, 4094)
N_S = N_MAIN + 1

# stripe-0 column chunks: chunk q feeds banks 2q, 2q+1
CHUNKS = [(0, 1026), (1024, 2050), (2048, 3074), (3072, 4096)]

_cached = None


def _build():
    nc = bass.Bass()
    f16 = mybir.dt.float16
    x_d = nc.dram_tensor("x", [IN_ROWS, W], f16, kind="ExternalInput")
    mb_d = nc.dram_tensor("mb", [128, KW * STRIPE], f16, kind="ExternalInput")
    mz_d = nc.dram_tensor("mz", [KW * 10, STUB_ROWS], f16, kind="ExternalInput")
    bv_d = nc.dram_tensor("bv", [128, 1], mybir.dt.float32, kind="ExternalInput")
    y_d = nc.dram_tensor("y", [CROWS, OW], f16, kind="ExternalOutput")

    import contextlib
    with contextlib.ExitStack() as st:
        ec = st.enter_context
        x0 = ec(nc.sbuf_tensor("x0", [128, W], f16))
        x1 = ec(nc.sbuf_tensor("x1", [128, W], f16))
        x2 = ec(nc.sbuf_tensor("x2", [128, W], f16))
        x3 = ec(nc.sbuf_tensor("x3", [128, W], f16))
        zb = ec(nc.sbuf_tensor("zb", [KW * 10, OW], f16))
        y0 = ec(nc.sbuf_tensor("y0", [128, OW], f16))
        y1 = ec(nc.sbuf_tensor("y1", [128, OW], f16))
        mb = ec(nc.sbuf_tensor("mb_sb", [128, KW * STRIPE], f16))
        mz = ec(nc.sbuf_tensor("mz_sb", [KW * 10, STUB_ROWS], f16))
        bv = ec(nc.sbuf_tensor("bv_sb", [128, 1], mybir.dt.float32))
        ps = ec(nc.psum_tensor([128, 4096], mybir.dt.float32))
        c_sem = ec(nc.semaphore("c_sem"))
        in0q = [ec(nc.semaphore(f"in0q{q}")) for q in range(4)]
        in1 = ec(nc.semaphore("in1"))
        in2 = ec(nc.semaphore("in2"))
        in3 = ec(nc.semaphore("in3"))
        pe_sem = ec(nc.semaphore("pe_sem"))
        ev_sem = ec(nc.semaphore("ev_sem"))   # scalar evacs (even banks), +1 each
        vec_sem = ec(nc.semaphore("vec_sem"))  # vector evacs (odd banks), +1 each
        st0 = ec(nc.semaphore("st0"))          # store-done, even stripes (+16/dma)
        st1 = ec(nc.semaphore("st1"))          # store-done, odd stripes
        blk = ec(nc.Block())

        xb = [x0, x1, x2, x3]
        yb = [y0, y1]
        ins = [None, in1, in2, in3]
        sts = [st0, st1]

        def bank_cols(b):
            c0 = 512 * b
            return c0, min(512, OW - c0)

        # stripe geometry: (out_row_start, out_rows, in_rows)
        def stripe_geo(s):
            if s < N_MAIN:
                return s * STRIPE, STRIPE, STRIPE + KH - 1
            return STUB_R0, STUB_ROWS, STUB_ROWS + KH - 1

        @blk.sync
        def _(sync):
            # stripe 0 in 4 column chunks so PE can start on bank 0 early
            for q, (cl, ch) in enumerate(CHUNKS):
                sync.dma_start(
                    xb[0][0:128, cl:ch], x_d.ap()[0:128, cl:ch]
                ).then_inc(in0q[q], 16)
            for s in range(1, N_MAIN):
                r0 = s * STRIPE
                sync.dma_start(
                    xb[s][0:128, :], x_d.ap()[r0:r0 + 128, :]
                ).then_inc(ins[s], 16)
            # hold the NEFF open until all outputs are stored
            n_even = (N_S + 1) // 2
            n_odd = N_S // 2
            sync.wait_ge(st0, 32 * n_even)
            sync.wait_ge(st1, 32 * n_odd)

        @blk.tensor
        def _(tensor):
            tensor.wait_ge(c_sem, 96)
            for s in range(N_S):
                r0, orows, irows = stripe_geo(s)
                if s >= 1 and s < N_MAIN:
                    tensor.wait_ge(ins[s], 16)
                for b in range(NBANKS):
                    c0, nb = bank_cols(b)
                    if s == 0 and b % 2 == 0:
                        tensor.wait_ge(in0q[b // 2], 16)
                    if s >= 1:
                        # previous stripe's bank b must be evacuated
                        if b % 2 == 0:
                            tensor.wait_ge(ev_sem, 4 * (s - 1) + b // 2 + 1)
                        else:
                            tensor.wait_ge(vec_sem, 4 * (s - 1) + (b - 1) // 2 + 1)
                    if s < N_MAIN:
                        xt = xb[s]
                        mm = None
                        for dj in range(KW):
                            mm = nc.tensor.matmul(
                                ps[0:orows, c0:c0 + nb],
                                mb[0:irows, dj * STRIPE:dj * STRIPE + orows],
                                xt[0:irows, c0 + dj:c0 + dj + nb],
                                start=(dj == 0),
                                stop=(dj == KW - 1),
                            )
                        mm.then_inc(pe_sem, 1)
                    else:
                        # stub: all 9 taps in one matmul against the
                        # dj-pre-shifted 30-partition tile
                        nc.tensor.matmul(
                            ps[0:orows, c0:c0 + nb],
                            mz[0:KW * 10, 0:orows],
                            zb[0:KW * 10, c0:c0 + nb],
                            start=True,
                            stop=True,
                        ).then_inc(pe_sem, 1)

        @blk.scalar
        def _(scalar):
            # constants + stub tile on scalar's HWDGE ring, in parallel with
            # sync's big input stream
            scalar.dma_start(mb[:], mb_d.ap()).then_inc(c_sem, 16)
            scalar.dma_start(bv[:], bv_d.ap()).then_inc(c_sem, 16)
            scalar.dma_start(mz[:], mz_d.ap()).then_inc(c_sem, 16)
            for dj in range(KW):
                scalar.dma_start(
                    zb[10 * dj:10 * dj + 10, 0:OW],
                    x_d.ap()[STUB_R0:STUB_R0 + 10, dj:dj + OW],
                ).then_inc(c_sem, 16)
            scalar.wait_ge(c_sem, 96)
            for s in range(N_S):
                r0, orows, irows = stripe_geo(s)
                yt = yb[s % 2]
                if s >= 2:
                    scalar.wait_ge(sts[s % 2], 32 * (s // 2))
                for b in (0, 2, 4, 6):
                    c0, nb = bank_cols(b)
                    scalar.wait_ge(pe_sem, NBANKS * s + b + 1)
                    nc.scalar.activation(
                        out=yt[0:orows, c0:c0 + nb],
                        in_=ps[0:orows, c0:c0 + nb],
                        func=mybir.ActivationFunctionType.Identity,
                        bias=bv[0:orows, 0:1],
                        scale=1.0,
                    ).then_inc(ev_sem, 1)
                # stores for this stripe; gated on both engines' evac sems
                # (HWDGE does not see compute writes)
                for h, (cl, ch) in enumerate(((0, HALF_COL), (HALF_COL, OW))):
                    scalar.wait_ge(ev_sem, 4 * s + 2 * (h + 1))
                    scalar.wait_ge(vec_sem, 4 * s + 2 * (h + 1))
                    scalar.dma_start(
                        y_d.ap()[r0:r0 + orows, cl:ch],
                        yt[0:orows, cl:ch],
                    ).then_inc(sts[s % 2], 16)

        @blk.vector
        def _(vector):
            vector.wait_ge(c_sem, 96)
            for s in range(N_S):
                r0, orows, irows = stripe_geo(s)
                yt = yb[s % 2]
                if s >= 2:
                    vector.wait_ge(sts[s % 2], 32 * (s // 2))
                for b in (1, 3, 5, 7):
                    c0, nb = bank_cols(b)
                    vector.wait_ge(pe_sem, NBANKS * s + b + 1)
                    nc.vector.tensor_scalar_add(
                        out=yt[0:orows, c0:c0 + nb],
                        in0=ps[0:orows, c0:c0 + nb],
                        scalar1=bv[0:orows, 0:1],
                    ).then_inc(vec_sem, 1)

    return nc


def _host_prep(input, weight, bias):
    input = np.ascontiguousarray(input, dtype=np.float32)
    weight = np.asarray(weight, dtype=np.float32)
    bias = np.asarray(bias, dtype=np.float32)
    w16 = weight.astype(np.float16)

    # band matrices packed side by side: mb[:, dj*126+m] column m of M_dj,
    # M_dj[k, m] = weight[k-m, dj] for 0 <= k-m < KH
    mb = np.zeros((128, KW * STRIPE), dtype=np.float16)
    idx = np.arange(STRIPE)
    for dj in range(KW):
        for di in range(KH):
            mb[idx + di, dj * STRIPE + idx] = w16[di, dj]

    # stub band: mz[10*dj + r, m] = w[r-m, dj] for 0 <= r-m < KH
    mz = np.zeros((KW * 10, STUB_ROWS), dtype=np.float16)
    for dj in range(KW):
        for m in range(STUB_ROWS):
            for di in range(KH):
                mz[10 * dj + m + di, m] = w16[di, dj]

    bv = np.full((128, 1), bias[0], dtype=np.float32)

    x16 = input.astype(np.float16)
    in_maps = []
    for i in range(NCORES):
        r0 = i * CROWS
        sl = x16[r0:r0 + IN_ROWS]
        if sl.shape[0] < IN_ROWS:
            sl = np.concatenate(
                [sl, np.zeros((IN_ROWS - sl.shape[0], W), np.float16)], axis=0
            )
        in_maps.append({"x": np.ascontiguousarray(sl), "mb": mb, "mz": mz, "bv": bv})
    return in_maps


def _run(input, weight, bias, **spmd_kwargs):
    global _cached
    if _cached is None:
        _cached = _build()
    in_maps = _host_prep(input, weight, bias)
    res = run_bass_kernel_spmd(
        _cached, in_maps, core_ids=list(range(NCORES)), **spmd_kwargs
    )
    out = np.empty((OH, OW), dtype=np.float32)
    for i in range(NCORES):
        r0 = i * CROWS
        rows = min(CROWS, OH - r0)
        out[r0:r0 + rows] = res.results[i]["y"][:rows].astype(np.float32)
    return out, res


def kernel(input, weight, bias):
    out, _ = _run(input, weight, bias)
    return out


# revision 10
# speedup vs baseline: 1.0083x; 1.0083x over previous
"""Trainium2 Bass kernel: 4096x4096 single-channel 3x3 VALID conv + bias.

Sharding: 8-way row-parallel. Core i computes output rows [512*i, 512*i+512)
(core 7: 510 valid rows). Halo handled host-side: each core's input shard is
[514, 4096] (512 rows + 2 halo rows; core 7 zero-padded).

v4: fp16 everywhere on device (tolerance 2e-2; fp16 end-to-end measures
~6.6e-4), halving HBM traffic. Host converts fp32<->fp16 around the run.

Per core: 4 main stripes of 126 output rows + an 8-row stub. A main stripe's
128 input rows sit on SBUF partitions; per 512-wide PSUM bank, 3 matmuls (one
per kernel column dj, rhs shifted by dj along the free dim) against 128x126
fp16 band matrices accumulate all 9 taps into fp32 PSUM. The stub instead
uses a [30, 4094] tile holding the last 10 input rows pre-shifted by dj
(3 extra tiny DRAM loads) and a [30, 8] band, so all 9 taps cost ONE matmul
per bank. All constants (bands + bias + stub band) ride in ONE [128, 387]
tensor = one DMA of 128 descriptors, so the PE isn't gated on a swarm of tiny
descriptors racing the input stream (that cost 8us in an earlier rev).
Stripe 0 is loaded in 4 column chunks so the PE can start after ~260KB; the
PE runs a few matmuls on a zeroed scratch tile first so the clock is ramped
(mid->max p-state takes ~3us) before real data lands. ScalarE (even banks) /
VectorE (odd banks) evacuate PSUM->SBUF fusing +bias and the fp32->fp16
cast; scalar's HWDGE ring stores each half stripe; sync's ring streams input.

Sync notes (hard-won):
- This walrus build allows at most ONE sem wait and ONE sem update per
  instruction; extra waits are standalone engine.wait_ge() instructions.
- An HWDGE dma_start on a compute engine's queue does NOT wait for prior
  compute writes to land; the DMA must be gated by a sem incremented by the
  last producing instruction, or it reads stale SBUF.
"""

import numpy as np

import concourse.bass as bass
import concourse.mybir as mybir
from concourse.bass_utils import run_bass_kernel_spmd

H = W = 4096
KH = KW = 3
OH = OW = H - KH + 1  # 4094
NCORES = 8
CROWS = 512            # output rows per core (core 7: 510 valid)
IN_ROWS = CROWS + KH - 1  # 514 input rows per core shard
STRIPE = 126           # output rows per full main stripe
N_MAIN = 4             # main stripes; stub covers rows [504, 512)
STUB_R0 = N_MAIN * STRIPE   # 504
STUB_ROWS = CROWS - STUB_R0  # 8
NBANKS = 8             # PSUM banks; bank b covers output cols [512b, 512b+Nb)
HALF_COL = 2048        # output halves: [0, 2048) and [2048, 4094)
N_S = N_MAIN + 1

# packed const layout: cols [0, 378) = 3 main bands, col 378 = bias,
# cols [379, 387) = stub band (partitions 0..29)
MC_BIAS = KW * STRIPE          # 378
MC_MZ = MC_BIAS + 1            # 379
MC_COLS = MC_MZ + STUB_ROWS    # 387

# stripe-0 column chunks: chunk q feeds banks 2q, 2q+1
CHUNKS = [(0, 1026), (1024, 2050), (2048, 3074), (3072, 4096)]

N_WARM = 7  # PE pre-warm matmuls on scratch

_cached = None


def _build():
    nc = bass.Bass()
    f16 = mybir.dt.float16
    x_d = nc.dram_tensor("x", [IN_ROWS, W], f16, kind="ExternalInput")
    mc_d = nc.dram_tensor("mc", [128, MC_COLS], f16, kind="ExternalInput")
    y_d = nc.dram_tensor("y", [CROWS, OW], f16, kind="ExternalOutput")

    import contextlib
    with contextlib.ExitStack() as st:
        ec = st.enter_context
        x0 = ec(nc.sbuf_tensor("x0", [128, W], f16))
        x1 = ec(nc.sbuf_tensor("x1", [128, W], f16))
        x2 = ec(nc.sbuf_tensor("x2", [128, W], f16))
        x3 = ec(nc.sbuf_tensor("x3", [128, W], f16))
        zb = ec(nc.sbuf_tensor("zb", [KW * 10, OW], f16))
        y0 = ec(nc.sbuf_tensor("y0", [128, OW], f16))
        y1 = ec(nc.sbuf_tensor("y1", [128, OW], f16))
        mc = ec(nc.sbuf_tensor("mc_sb", [128, MC_COLS], f16))
        bv32 = ec(nc.sbuf_tensor("bv32", [128, 1], mybir.dt.float32))
        wsc = ec(nc.sbuf_tensor("wsc", [128, 640], f16))  # PE warm-up scratch
        ps = ec(nc.psum_tensor([128, 4096], mybir.dt.float32))
        cm_sem = ec(nc.semaphore("cm_sem"))
        cz_sem = ec(nc.semaphore("cz_sem"))
        zz_sem = ec(nc.semaphore("zz_sem"))
        bvs = ec(nc.semaphore("bvs"))
        in0q = [ec(nc.semaphore(f"in0q{q}")) for q in range(4)]
        in1 = ec(nc.semaphore("in1"))
        in2 = ec(nc.semaphore("in2"))
        in3 = ec(nc.semaphore("in3"))
        pe_sem = ec(nc.semaphore("pe_sem"))
        ev_sem = ec(nc.semaphore("ev_sem"))   # scalar evacs (even banks), +1 each
        vec_sem = ec(nc.semaphore("vec_sem"))  # vector evacs (odd banks), +1 each
        st0 = ec(nc.semaphore("st0"))          # store-done, even stripes (+16/dma)
        st1 = ec(nc.semaphore("st1"))          # store-done, odd stripes
        blk = ec(nc.Block())

        xb = [x0, x1, x2, x3]
        yb = [y0, y1]
        ins = [None, in1, in2, in3]
        sts = [st0, st1]

        def bank_cols(b):
            c0 = 512 * b
            return c0, min(512, OW - c0)

        # stripe geometry: (out_row_start, out_rows, in_rows)
        def stripe_geo(s):
            if s < N_MAIN:
                return s * STRIPE, STRIPE, STRIPE + KH - 1
            return STUB_R0, STUB_ROWS, STUB_ROWS + KH - 1

        @blk.sync
        def _(sync):
            # stripe 0 in 4 column chunks so PE can start on bank 0 early
            for q, (cl, ch) in enumerate(CHUNKS):
                sync.dma_start(
                    xb[0][0:128, cl:ch], x_d.ap()[0:128, cl:ch]
                ).then_inc(in0q[q], 16)
            for s in range(1, N_MAIN):
                r0 = s * STRIPE
                sync.dma_start(
                    xb[s][0:128, :], x_d.ap()[r0:r0 + 128, :]
                ).then_inc(ins[s], 16)
            # hold the NEFF open until all outputs are stored
            n_even = (N_S + 1) // 2
            n_odd = N_S // 2
            sync.wait_ge(st0, 32 * n_even)
            sync.wait_ge(st1, 32 * n_odd)

        @blk.gpsimd
        def _(gpsimd):
            gpsimd.memset(wsc[:, :], 0.0).then_inc(zz_sem, 1)

        @blk.tensor
        def _(tensor):
            # pre-warm the PE p-state on zeroed scratch while inputs stream in
            tensor.wait_ge(zz_sem, 1)
            for _w in range(N_WARM):
                nc.tensor.matmul(
                    ps[0:126, 0:512],
                    wsc[0:128, 0:126],
                    wsc[0:128, 128:640],
                    start=True,
                    stop=True,
                )
            tensor.wait_ge(cm_sem, 16)
            for s in range(N_S):
                r0, orows, irows = stripe_geo(s)
                if s >= 1 and s < N_MAIN:
                    tensor.wait_ge(ins[s], 16)
                if s == N_MAIN:
                    tensor.wait_ge(cz_sem, 48)
                for b in range(NBANKS):
                    c0, nb = bank_cols(b)
                    if s == 0 and b % 2 == 0:
                        tensor.wait_ge(in0q[b // 2], 16)
                    if s >= 1:
                        # previous stripe's bank b must be evacuated
                        if b % 2 == 0:
                            tensor.wait_ge(ev_sem, 4 * (s - 1) + b // 2 + 1)
                        else:
                            tensor.wait_ge(vec_sem, 4 * (s - 1) + (b - 1) // 2 + 1)
                    if s < N_MAIN:
                        xt = xb[s]
                        mm = None
                        for dj in range(KW):
                            mm = nc.tensor.matmul(
                                ps[0:orows, c0:c0 + nb],
                                mc[0:irows, dj * STRIPE:dj * STRIPE + orows],
                                xt[0:irows, c0 + dj:c0 + dj + nb],
                                start=(dj == 0),
                                stop=(dj == KW - 1),
                            )
                        mm.then_inc(pe_sem, 1)
                    else:
                        # stub: all 9 taps in one matmul against the
                        # dj-pre-shifted 30-partition tile
                        nc.tensor.matmul(
                            ps[0:orows, c0:c0 + nb],
                            mc[0:KW * 10, MC_MZ:MC_MZ + orows],
                            zb[0:KW * 10, c0:c0 + nb],
                            start=True,
                            stop=True,
                        ).then_inc(pe_sem, 1)

        @blk.scalar
        def _(scalar):
            # constants + stub tile on scalar's HWDGE ring, in parallel with
            # sync's big input stream
            scalar.dma_start(mc[:], mc_d.ap()).then_inc(cm_sem, 16)
            for dj in range(KW):
                scalar.dma_start(
                    zb[10 * dj:10 * dj + 10, 0:OW],
                    x_d.ap()[STUB_R0:STUB_R0 + 10, dj:dj + OW],
                ).then_inc(cz_sem, 16)
            scalar.wait_ge(bvs, 1)
            for s in range(N_S):
                r0, orows, irows = stripe_geo(s)
                yt = yb[s % 2]
                if s >= 2:
                    scalar.wait_ge(sts[s % 2], 32 * (s // 2))
                for b in (0, 2, 4, 6):
                    c0, nb = bank_cols(b)
                    scalar.wait_ge(pe_sem, NBANKS * s + b + 1)
                    nc.scalar.activation(
                        out=yt[0:orows, c0:c0 + nb],
                        in_=ps[0:orows, c0:c0 + nb],
                        func=mybir.ActivationFunctionType.Identity,
                        bias=bv32[0:orows, 0:1],
                        scale=1.0,
                    ).then_inc(ev_sem, 1)
                # stores for this stripe; gated on both engines' evac sems
                # (HWDGE does not see compute writes)
                for h, (cl, ch) in enumerate(((0, HALF_COL), (HALF_COL, OW))):
                    scalar.wait_ge(ev_sem, 4 * s + 2 * (h + 1))
                    scalar.wait_ge(vec_sem, 4 * s + 2 * (h + 1))
                    scalar.dma_start(
                        y_d.ap()[r0:r0 + orows, cl:ch],
                        yt[0:orows, cl:ch],
                    ).then_inc(sts[s % 2], 16)

        @blk.vector
        def _(vector):
            vector.wait_ge(cm_sem, 16)
            # widen the fp16 bias column to fp32 once (engine scalar operands
            # must be fp32)
            nc.vector.tensor_copy(
                out=bv32[0:128, 0:1], in_=mc[0:128, MC_BIAS:MC_BIAS + 1]
            ).then_inc(bvs, 1)
            for s in range(N_S):
                r0, orows, irows = stripe_geo(s)
                yt = yb[s % 2]
                if s >= 2:
                    vector.wait_ge(sts[s % 2], 32 * (s // 2))
                for b in (1, 3, 5, 7):
                    c0, nb = bank_cols(b)
                    vector.wait_ge(pe_sem, NBANKS * s + b + 1)
                    nc.vector.tensor_scalar_add(
                        out=yt[0:orows, c0:c0 + nb],
                        in0=ps[0:orows, c0:c0 + nb],
                        scalar1=bv32[0:orows, 0:1],
                    ).then_inc(vec_sem, 1)

    return nc


def _host_prep(input, weight, bias):
    input = np.ascontiguousarray(input, dtype=np.float32)
    weight = np.asarray(weight, dtype=np.float32)
    bias = np.asarray(bias, dtype=np.float32)
    w16 = weight.astype(np.float16)

    mc = np.zeros((128, MC_COLS), dtype=np.float16)
    # band matrices packed side by side: mc[:, dj*126+m] column m of M_dj,
    # M_dj[k, m] = weight[k-m, dj] for 0 <= k-m < KH
    idx = np.arange(STRIPE)
    for dj in range(KW):
        for di in range(KH):
            mc[idx + di, dj * STRIPE + idx] = w16[di, dj]
    # bias column (fp16; |err| <= 2^-11*|b|, well within tolerance)
    mc[:, MC_BIAS] = np.float16(bias[0])
    # stub band: mc[10*dj + m + di, MC_MZ + m] = w[di, dj]
    for dj in range(KW):
        for m in range(STUB_ROWS):
            for di in range(KH):
                mc[10 * dj + m + di, MC_MZ + m] = w16[di, dj]

    x16 = input.astype(np.float16)
    in_maps = []
    for i in range(NCORES):
        r0 = i * CROWS
        sl = x16[r0:r0 + IN_ROWS]
        if sl.shape[0] < IN_ROWS:
            sl = np.concatenate(
                [sl, np.zeros((IN_ROWS - sl.shape[0], W), np.float16)], axis=0
            )
        in_maps.append({"x": np.ascontiguousarray(sl), "mc": mc})
    return in_maps


def _run(input, weight, bias, **spmd_kwargs):
    global _cached
    if _cached is None:
        _cached = _build()
    in_maps = _host_prep(input, weight, bias)
    res = run_bass_kernel_spmd(
        _cached, in_maps, core_ids=list(range(NCORES)), **spmd_kwargs
    )
    out = np.empty((OH, OW), dtype=np.float32)
    for i in range(NCORES):
        r0 = i * CROWS
        rows = min(CROWS, OH - r0)
        out[r0:r0 + rows] = res.results[i]["y"][:rows].astype(np.float32)
    return out, res


def kernel(input, weight, bias):
    out, _ = _run(input, weight, bias)
    return out


# revision 15
# speedup vs baseline: 1.1969x; 1.1870x over previous
"""Trainium2 Bass kernel: 4096x4096 single-channel 3x3 VALID conv + bias.

Sharding: 8-way row-parallel. Core i computes output rows [512*i, 512*i+512)
(core 7: 510 valid rows). Halo handled host-side: each core's input shard is
[514, 4096] (512 rows + 2 halo rows; core 7 zero-padded).

v4: fp16 everywhere on device (tolerance 2e-2; fp16 end-to-end measures
~6.6e-4), halving HBM traffic. Host converts fp32<->fp16 around the run.

Per core: 4 main stripes of 126 output rows + an 8-row stub. A main stripe's
128 input rows sit on SBUF partitions; per 512-wide PSUM bank, 3 matmuls (one
per kernel column dj, rhs shifted by dj along the free dim) against 128x126
fp16 band matrices accumulate all 9 taps into fp32 PSUM. The stub instead
uses a [30, 4094] tile holding the last 10 input rows pre-shifted by dj
(3 extra tiny DRAM loads) and a [30, 8] band, so all 9 taps cost ONE matmul
per bank. All constants (bands + bias + stub band) ride in ONE [128, 387]
tensor = one DMA of 128 descriptors, so the PE isn't gated on a swarm of tiny
descriptors racing the input stream (that cost 8us in an earlier rev).
Stripe 0 is loaded in 4 column chunks so the PE can start after ~260KB; the
PE runs a few matmuls on a zeroed scratch tile first so the clock is ramped
(mid->max p-state takes ~3us) before real data lands. ScalarE (even banks) /
VectorE (odd banks) evacuate PSUM->SBUF fusing +bias and the fp32->fp16
cast; scalar's HWDGE ring stores each half stripe; sync's ring streams input.

Sync notes (hard-won):
- This walrus build allows at most ONE sem wait and ONE sem update per
  instruction; extra waits are standalone engine.wait_ge() instructions.
- An HWDGE dma_start on a compute engine's queue does NOT wait for prior
  compute writes to land; the DMA must be gated by a sem incremented by the
  last producing instruction, or it reads stale SBUF.
"""

import numpy as np

import concourse.bass as bass
import concourse.mybir as mybir
from concourse.bass_utils import run_bass_kernel_spmd

H = W = 4096
KH = KW = 3
OH = OW = H - KH + 1  # 4094
NCORES = 8
CROWS = 512            # output rows per core (core 7: 510 valid)
IN_ROWS = CROWS + KH - 1  # 514 input rows per core shard
STRIPE = 126           # output rows per full main stripe
N_MAIN = 4             # main stripes; stub covers rows [504, 512)
STUB_R0 = N_MAIN * STRIPE   # 504
STUB_ROWS = CROWS - STUB_R0  # 8
NBANKS = 8             # PSUM banks; bank b covers output cols [512b, 512b+Nb)
HALF_COL = 2048        # output halves: [0, 2048) and [2048, 4094)
N_S = N_MAIN + 1

# packed const layout: cols [0, 378) = 3 main bands, col 378 = bias,
# cols [379, 387) = stub band (partitions 0..29)
MC_BIAS = KW * STRIPE          # 378
MC_MZ = MC_BIAS + 1            # 379
MC_COLS = MC_MZ + STUB_ROWS    # 387

# stripe-0 column halves: half 0 feeds banks 0-3, half 1 feeds banks 4-7
# (4KB+ descriptors; smaller ones run far below DMA peak rate)
CHUNKS = [(0, 2050), (2048, 4096)]

N_WARM = 10  # PE pre-warm matmuls on scratch (keeps clock ramped until data)

_cached = None


def _build():
    nc = bass.Bass()
    f16 = mybir.dt.float16
    x_d = nc.dram_tensor("x", [IN_ROWS, W], f16, kind="ExternalInput")
    mc_d = nc.dram_tensor("mc", [128, MC_COLS], f16, kind="ExternalInput")
    y_d = nc.dram_tensor("y", [CROWS, OW], f16, kind="ExternalOutput")

    import contextlib
    with contextlib.ExitStack() as st:
        ec = st.enter_context
        x0 = ec(nc.sbuf_tensor("x0", [128, W], f16))
        x1 = ec(nc.sbuf_tensor("x1", [128, W], f16))
        x2 = ec(nc.sbuf_tensor("x2", [128, W], f16))
        x3 = ec(nc.sbuf_tensor("x3", [128, W], f16))
        zb = ec(nc.sbuf_tensor("zb", [KW * 10, OW], f16))
        y0 = ec(nc.sbuf_tensor("y0", [128, OW], f16))
        y1 = ec(nc.sbuf_tensor("y1", [128, OW], f16))
        mc = ec(nc.sbuf_tensor("mc_sb", [128, MC_COLS], f16))
        bv32 = ec(nc.sbuf_tensor("bv32", [128, 1], mybir.dt.float32))
        wsc = ec(nc.sbuf_tensor("wsc", [128, 640], f16))  # PE warm-up scratch
        ps = ec(nc.psum_tensor([128, 4096], mybir.dt.float32))
        cm_sem = ec(nc.semaphore("cm_sem"))
        cz_sem = ec(nc.semaphore("cz_sem"))
        zz_sem = ec(nc.semaphore("zz_sem"))
        bvs = ec(nc.semaphore("bvs"))
        in0q = [ec(nc.semaphore(f"in0q{q}")) for q in range(len(CHUNKS))]
        in1 = ec(nc.semaphore("in1"))
        in2 = ec(nc.semaphore("in2"))
        in3 = ec(nc.semaphore("in3"))
        pe_sem = ec(nc.semaphore("pe_sem"))
        ev_sem = ec(nc.semaphore("ev_sem"))   # scalar evacs (even banks), +1 each
        vec_sem = ec(nc.semaphore("vec_sem"))  # vector evacs (odd banks), +1 each
        st0 = ec(nc.semaphore("st0"))          # store-done, even stripes (+16/dma)
        st1 = ec(nc.semaphore("st1"))          # store-done, odd stripes
        blk = ec(nc.Block())

        xb = [x0, x1, x2, x3]
        yb = [y0, y1]
        ins = [None, in1, in2, in3]
        sts = [st0, st1]

        def bank_cols(b):
            c0 = 512 * b
            return c0, min(512, OW - c0)

        # stripe geometry: (out_row_start, out_rows, in_rows)
        def stripe_geo(s):
            if s < N_MAIN:
                return s * STRIPE, STRIPE, STRIPE + KH - 1
            return STUB_R0, STUB_ROWS, STUB_ROWS + KH - 1

        @blk.sync
        def _(sync):
            # the packed consts gate the PE — issue them first on this ring
            sync.dma_start(mc[:], mc_d.ap()).then_inc(cm_sem, 16)
            # stripe 0 in column halves so PE can start on banks 0-3 early
            for q, (cl, ch) in enumerate(CHUNKS):
                sync.dma_start(
                    xb[0][0:128, cl:ch], x_d.ap()[0:128, cl:ch]
                ).then_inc(in0q[q], 16)
            for s in range(1, N_MAIN):
                r0 = s * STRIPE
                sync.dma_start(
                    xb[s][0:128, :], x_d.ap()[r0:r0 + 128, :]
                ).then_inc(ins[s], 16)
            # hold the NEFF open until all outputs are stored
            n_even = (N_S + 1) // 2
            n_odd = N_S // 2
            sync.wait_ge(st0, 32 * n_even)
            sync.wait_ge(st1, 32 * n_odd)

        @blk.gpsimd
        def _(gpsimd):
            gpsimd.memset(wsc[:, :], 0.0).then_inc(zz_sem, 1)

        @blk.tensor
        def _(tensor):
            # pre-warm the PE p-state on zeroed scratch while inputs stream in
            tensor.wait_ge(zz_sem, 1)
            for _w in range(N_WARM):
                nc.tensor.matmul(
                    ps[0:126, 0:512],
                    wsc[0:128, 0:126],
                    wsc[0:128, 128:640],
                    start=True,
                    stop=True,
                )
            tensor.wait_ge(cm_sem, 16)
            for s in range(N_S):
                r0, orows, irows = stripe_geo(s)
                if s >= 1 and s < N_MAIN:
                    tensor.wait_ge(ins[s], 16)
                if s == N_MAIN:
                    tensor.wait_ge(cz_sem, 48)
                for b in range(NBANKS):
                    c0, nb = bank_cols(b)
                    if s == 0 and b % 4 == 0:
                        tensor.wait_ge(in0q[b // 4], 16)
                    if s >= 1:
                        # previous stripe's bank b must be evacuated
                        if b % 2 == 0:
                            tensor.wait_ge(ev_sem, 4 * (s - 1) + b // 2 + 1)
                        else:
                            tensor.wait_ge(vec_sem, 4 * (s - 1) + (b - 1) // 2 + 1)
                    if s < N_MAIN:
                        xt = xb[s]
                        mm = None
                        for dj in range(KW):
                            mm = nc.tensor.matmul(
                                ps[0:orows, c0:c0 + nb],
                                mc[0:irows, dj * STRIPE:dj * STRIPE + orows],
                                xt[0:irows, c0 + dj:c0 + dj + nb],
                                start=(dj == 0),
                                stop=(dj == KW - 1),
                            )
                        mm.then_inc(pe_sem, 1)
                    else:
                        # stub: all 9 taps in one matmul against the
                        # dj-pre-shifted 30-partition tile
                        nc.tensor.matmul(
                            ps[0:orows, c0:c0 + nb],
                            mc[0:KW * 10, MC_MZ:MC_MZ + orows],
                            zb[0:KW * 10, c0:c0 + nb],
                            start=True,
                            stop=True,
                        ).then_inc(pe_sem, 1)

        @blk.scalar
        def _(scalar):
            # stub tile on scalar's HWDGE ring, in parallel with sync's stream
            for dj in range(KW):
                scalar.dma_start(
                    zb[10 * dj:10 * dj + 10, 0:OW],
                    x_d.ap()[STUB_R0:STUB_R0 + 10, dj:dj + OW],
                ).then_inc(cz_sem, 16)
            scalar.wait_ge(bvs, 1)
            for s in range(N_S):
                r0, orows, irows = stripe_geo(s)
                yt = yb[s % 2]
                if s >= 2:
                    scalar.wait_ge(sts[s % 2], 32 * (s // 2))
                for b in (0, 2, 4, 6):
                    c0, nb = bank_cols(b)
                    scalar.wait_ge(pe_sem, NBANKS * s + b + 1)
                    nc.scalar.activation(
                        out=yt[0:orows, c0:c0 + nb],
                        in_=ps[0:orows, c0:c0 + nb],
                        func=mybir.ActivationFunctionType.Identity,
                        bias=bv32[0:orows, 0:1],
                        scale=1.0,
                    ).then_inc(ev_sem, 1)
                # stores for this stripe; gated on both engines' evac sems
                # (HWDGE does not see compute writes)
                for h, (cl, ch) in enumerate(((0, HALF_COL), (HALF_COL, OW))):
                    scalar.wait_ge(ev_sem, 4 * s + 2 * (h + 1))
                    scalar.wait_ge(vec_sem, 4 * s + 2 * (h + 1))
                    scalar.dma_start(
                        y_d.ap()[r0:r0 + orows, cl:ch],
                        yt[0:orows, cl:ch],
                    ).then_inc(sts[s % 2], 16)

        @blk.vector
        def _(vector):
            vector.wait_ge(cm_sem, 16)
            # widen the fp16 bias column to fp32 once (engine scalar operands
            # must be fp32)
            nc.vector.tensor_copy(
                out=bv32[0:128, 0:1], in_=mc[0:128, MC_BIAS:MC_BIAS + 1]
            ).then_inc(bvs, 1)
            for s in range(N_S):
                r0, orows, irows = stripe_geo(s)
                yt = yb[s % 2]
                if s >= 2:
                    vector.wait_ge(sts[s % 2], 32 * (s // 2))
                for b in (1, 3, 5, 7):
                    c0, nb = bank_cols(b)
                    vector.wait_ge(pe_sem, NBANKS * s + b + 1)
                    nc.vector.tensor_scalar_add(
                        out=yt[0:orows, c0:c0 + nb],
                        in0=ps[0:orows, c0:c0 + nb],
                        scalar1=bv32[0:orows, 0:1],
                    ).then_inc(vec_sem, 1)

    return nc


def _host_prep(input, weight, bias):
    input = np.ascontiguousarray(input, dtype=np.float32)
    weight = np.asarray(weight, dtype=np.float32)
    bias = np.asarray(bias, dtype=np.float32)
    w16 = weight.astype(np.float16)

    mc = np.zeros((128, MC_COLS), dtype=np.float16)
    # band matrices packed side by side: mc[:, dj*126+m] column m of M_dj,
    # M_dj[k, m] = weight[k-m, dj] for 0 <= k-m < KH
    idx = np.arange(STRIPE)
    for dj in range(KW):
        for di in range(KH):
            mc[idx + di, dj * STRIPE + idx] = w16[di, dj]
    # bias column (fp16; |err| <= 2^-11*|b|, well within tolerance)
    mc[:, MC_BIAS] = np.float16(bias[0])
    # stub band: mc[10*dj + m + di, MC_MZ + m] = w[di, dj]
    for dj in range(KW):
        for m in range(STUB_ROWS):
            for di in range(KH):
                mc[10 * dj + m + di, MC_MZ + m] = w16[di, dj]

    x16 = input.astype(np.float16)
    in_maps = []
    for i in range(NCORES):
        r0 = i * CROWS
        sl = x16[r0:r0 + IN_ROWS]
        if sl.shape[0] < IN_ROWS:
            sl = np.concatenate(
                [sl, np.zeros((IN_ROWS - sl.shape[0], W), np.float16)], axis=0
            )
        in_maps.append({"x": np.ascontiguousarray(sl), "mc": mc})
    return in_maps


def _run(input, weight, bias, **spmd_kwargs):
    global _cached
    if _cached is None:
        _cached = _build()
    in_maps = _host_prep(input, weight, bias)
    res = run_bass_kernel_spmd(
        _cached, in_maps, core_ids=list(range(NCORES)), **spmd_kwargs
    )
    out = np.empty((OH, OW), dtype=np.float32)
    for i in range(NCORES):
        r0 = i * CROWS
        rows = min(CROWS, OH - r0)
        out[r0:r0 + rows] = res.results[i]["y"][:rows].astype(np.float32)
    return out, res


def kernel(input, weight, bias):
    out, _ = _run(input, weight, bias)
    return out


# revision 20
# speedup vs baseline: 1.2052x; 1.0069x over previous
"""Trainium2 Bass kernel: 4096x4096 single-channel 3x3 VALID conv + bias.

Sharding: 8-way row-parallel. Core i computes output rows [512*i, 512*i+512)
(core 7: 510 valid rows). Halo handled host-side: each core's input shard is
[514, 4096] (512 rows + 2 halo rows; core 7 zero-padded).

v4: fp16 everywhere on device (tolerance 2e-2; fp16 end-to-end measures
~6.6e-4), halving HBM traffic. Host converts fp32<->fp16 around the run.

Per core: 4 main stripes of 126 output rows + an 8-row stub. A main stripe's
128 input rows sit on SBUF partitions; per 512-wide PSUM bank, 3 matmuls (one
per kernel column dj, rhs shifted by dj along the free dim) against 128x126
fp16 band matrices accumulate all 9 taps into fp32 PSUM. The stub instead
uses a [30, 4094] tile holding the last 10 input rows pre-shifted by dj
(3 extra tiny DRAM loads) and a [30, 8] band, so all 9 taps cost ONE matmul
per bank. All constants (bands + bias + stub band) ride in ONE [128, 387]
tensor = one DMA of 128 descriptors, so the PE isn't gated on a swarm of tiny
descriptors racing the input stream (that cost 8us in an earlier rev).
Stripe 0 is loaded in 4 column chunks so the PE can start after ~260KB; the
PE runs a few matmuls on a zeroed scratch tile first so the clock is ramped
(mid->max p-state takes ~3us) before real data lands. ScalarE (even banks) /
VectorE (odd banks) evacuate PSUM->SBUF fusing +bias and the fp32->fp16
cast; scalar's HWDGE ring stores each half stripe; sync's ring streams input.

Sync notes (hard-won):
- This walrus build allows at most ONE sem wait and ONE sem update per
  instruction; extra waits are standalone engine.wait_ge() instructions.
- An HWDGE dma_start on a compute engine's queue does NOT wait for prior
  compute writes to land; the DMA must be gated by a sem incremented by the
  last producing instruction, or it reads stale SBUF.
"""

import numpy as np

import concourse.bass as bass
import concourse.mybir as mybir
from concourse.bass_utils import run_bass_kernel_spmd

H = W = 4096
KH = KW = 3
OH = OW = H - KH + 1  # 4094
NCORES = 8
CROWS = 512            # output rows per core (core 7: 510 valid)
IN_ROWS = CROWS + KH - 1  # 514 input rows per core shard
STRIPE = 126           # output rows per full main stripe
N_MAIN = 4             # main stripes; stub covers rows [504, 512)
STUB_R0 = N_MAIN * STRIPE   # 504
STUB_ROWS = CROWS - STUB_R0  # 8
NBANKS = 8             # PSUM banks; bank b covers output cols [512b, 512b+Nb)
HALF_COL = 2048        # output halves: [0, 2048) and [2048, 4094)
N_S = N_MAIN + 1

# packed const layout: cols [0, 378) = 3 main bands, col 378 = bias,
# cols [379, 387) = stub band (partitions 0..29)
MC_BIAS = KW * STRIPE          # 378
MC_MZ = MC_BIAS + 1            # 379
MC_COLS = MC_MZ + STUB_ROWS    # 387

# stripe-0 column halves: half 0 feeds banks 0-3, half 1 feeds banks 4-7
# (4KB+ descriptors; smaller ones run far below DMA peak rate)
CHUNKS = [(0, 2050), (2048, 4096)]

N_WARM = 3  # PE pre-warm matmuls on scratch, gated on the const load

_cached = None


def _build():
    nc = bass.Bass()
    f16 = mybir.dt.float16
    x_d = nc.dram_tensor("x", [IN_ROWS, W], f16, kind="ExternalInput")
    mc_d = nc.dram_tensor("mc", [128, MC_COLS], f16, kind="ExternalInput")
    y_d = nc.dram_tensor("y", [CROWS, OW], f16, kind="ExternalOutput")

    import contextlib
    with contextlib.ExitStack() as st:
        ec = st.enter_context
        x0 = ec(nc.sbuf_tensor("x0", [128, W], f16))
        x1 = ec(nc.sbuf_tensor("x1", [128, W], f16))
        x2 = ec(nc.sbuf_tensor("x2", [128, W], f16))
        x3 = ec(nc.sbuf_tensor("x3", [128, W], f16))
        zb = ec(nc.sbuf_tensor("zb", [KW * 10, OW], f16))
        y0 = ec(nc.sbuf_tensor("y0", [128, OW], f16))
        y1 = ec(nc.sbuf_tensor("y1", [128, OW], f16))
        mc = ec(nc.sbuf_tensor("mc_sb", [128, MC_COLS], f16))
        bv32 = ec(nc.sbuf_tensor("bv32", [128, 1], mybir.dt.float32))
        wsc = ec(nc.sbuf_tensor("wsc", [128, 640], f16))  # PE warm-up scratch
        ps = ec(nc.psum_tensor([128, 4096], mybir.dt.float32))
        cm_sem = ec(nc.semaphore("cm_sem"))
        cz_sem = ec(nc.semaphore("cz_sem"))
        zz_sem = ec(nc.semaphore("zz_sem"))
        bvs = ec(nc.semaphore("bvs"))
        in0q = [ec(nc.semaphore(f"in0q{q}")) for q in range(len(CHUNKS))]
        in1 = ec(nc.semaphore("in1"))
        in2 = ec(nc.semaphore("in2"))
        in3 = ec(nc.semaphore("in3"))
        pe_sem = ec(nc.semaphore("pe_sem"))
        ev_sem = ec(nc.semaphore("ev_sem"))   # scalar evacs (even banks), +1 each
        vec_sem = ec(nc.semaphore("vec_sem"))  # vector evacs (odd banks), +1 each
        st0 = ec(nc.semaphore("st0"))          # store-done, even stripes (+16/dma)
        st1 = ec(nc.semaphore("st1"))          # store-done, odd stripes
        blk = ec(nc.Block())

        xb = [x0, x1, x2, x3]
        yb = [y0, y1]
        ins = [None, in1, in2, in3]
        sts = [st0, st1]

        def bank_cols(b):
            c0 = 512 * b
            return c0, min(512, OW - c0)

        # stripe geometry: (out_row_start, out_rows, in_rows)
        def stripe_geo(s):
            if s < N_MAIN:
                return s * STRIPE, STRIPE, STRIPE + KH - 1
            return STUB_R0, STUB_ROWS, STUB_ROWS + KH - 1

        @blk.sync
        def _(sync):
            # the packed consts gate the PE — issue them first on this ring
            sync.dma_start(mc[:], mc_d.ap()).then_inc(cm_sem, 16)
            # stripe 0 in column halves so PE can start on banks 0-3 early
            for q, (cl, ch) in enumerate(CHUNKS):
                sync.dma_start(
                    xb[0][0:128, cl:ch], x_d.ap()[0:128, cl:ch]
                ).then_inc(in0q[q], 16)
            for s in range(1, N_MAIN):
                r0 = s * STRIPE
                sync.dma_start(
                    xb[s][0:128, :], x_d.ap()[r0:r0 + 128, :]
                ).then_inc(ins[s], 16)
            # hold the NEFF open until all outputs are stored
            n_even = (N_S + 1) // 2
            n_odd = N_S // 2
            sync.wait_ge(st0, 32 * n_even)
            sync.wait_ge(st1, 32 * n_odd)

        @blk.gpsimd
        def _(gpsimd):
            gpsimd.memset(wsc[:, :], 0.0).then_inc(zz_sem, 1)
            # all stores ride the software DGE: dma_start costs ~25ns here vs
            # ~800ns of DIRECT2D occupancy on an HWDGE sequencer
            for s in range(N_S):
                r0, orows, irows = stripe_geo(s)
                yt = yb[s % 2]
                for h, (cl, ch) in enumerate(((0, HALF_COL), (HALF_COL, OW))):
                    # half 0 is written only by scalar's evacs, half 1 only by
                    # vector's
                    gpsimd.wait_ge((ev_sem, vec_sem)[h], 2 * s + 2)
                    gpsimd.dma_start(
                        y_d.ap()[r0:r0 + orows, cl:ch],
                        yt[0:orows, cl:ch],
                    ).then_inc(sts[s % 2], 16)

        @blk.tensor
        def _(tensor):
            tensor.wait_ge(zz_sem, 1)
            tensor.wait_ge(cm_sem, 16)
            # pre-warm the PE p-state on zeroed scratch while inputs stream in
            for _w in range(N_WARM):
                nc.tensor.matmul(
                    ps[0:126, 0:512],
                    wsc[0:128, 0:126],
                    wsc[0:128, 128:640],
                    start=True,
                    stop=True,
                )
            for s in range(N_S):
                r0, orows, irows = stripe_geo(s)
                if s >= 1 and s < N_MAIN:
                    tensor.wait_ge(ins[s], 16)
                if s == N_MAIN:
                    tensor.wait_ge(cz_sem, 48)
                for b in range(NBANKS):
                    c0, nb = bank_cols(b)
                    if s == 0 and b % 4 == 0:
                        tensor.wait_ge(in0q[b // 4], 16)
                    if s >= 1 and b % 2 == 0:
                        # previous stripe's bank pair must be evacuated
                        # (scalar owns cols [0, 2048) = banks 0-3 in 2 quarter
                        # instrs; vector owns [2048, 4094) = banks 4-7)
                        if b < 4:
                            tensor.wait_ge(ev_sem, 2 * (s - 1) + b // 2 + 1)
                        else:
                            tensor.wait_ge(vec_sem, 2 * (s - 1) + (b - 4) // 2 + 1)
                    if s < N_MAIN:
                        xt = xb[s]
                        mm = None
                        for dj in range(KW):
                            mm = nc.tensor.matmul(
                                ps[0:orows, c0:c0 + nb],
                                mc[0:irows, dj * STRIPE:dj * STRIPE + orows],
                                xt[0:irows, c0 + dj:c0 + dj + nb],
                                start=(dj == 0),
                                stop=(dj == KW - 1),
                            )
                        mm.then_inc(pe_sem, 1)
                    else:
                        # stub: all 9 taps in one matmul against the
                        # dj-pre-shifted 30-partition tile
                        nc.tensor.matmul(
                            ps[0:orows, c0:c0 + nb],
                            mc[0:KW * 10, MC_MZ:MC_MZ + orows],
                            zb[0:KW * 10, c0:c0 + nb],
                            start=True,
                            stop=True,
                        ).then_inc(pe_sem, 1)

        @blk.scalar
        def _(scalar):
            # stub tile on scalar's HWDGE ring, in parallel with sync's stream
            for dj in range(KW):
                scalar.dma_start(
                    zb[10 * dj:10 * dj + 10, 0:OW],
                    x_d.ap()[STUB_R0:STUB_R0 + 10, dj:dj + OW],
                ).then_inc(cz_sem, 16)
            scalar.wait_ge(bvs, 1)
            for s in range(N_S):
                r0, orows, irows = stripe_geo(s)
                yt = yb[s % 2]
                if s >= 2:
                    scalar.wait_ge(sts[s % 2], 32 * (s // 2))
                for q, (cl, ch) in enumerate(((0, 1024), (1024, 2048))):
                    # quarter q covers banks 2q, 2q+1
                    scalar.wait_ge(pe_sem, NBANKS * s + 2 * q + 2)
                    nc.scalar.activation(
                        out=yt[0:orows, cl:ch],
                        in_=ps[0:orows, cl:ch],
                        func=mybir.ActivationFunctionType.Identity,
                        bias=bv32[0:orows, 0:1],
                        scale=1.0,
                    ).then_inc(ev_sem, 1)

        @blk.vector
        def _(vector):
            vector.wait_ge(cm_sem, 16)
            # widen the fp16 bias column to fp32 once (engine scalar operands
            # must be fp32)
            nc.vector.tensor_copy(
                out=bv32[0:128, 0:1], in_=mc[0:128, MC_BIAS:MC_BIAS + 1]
            ).then_inc(bvs, 1)
            for s in range(N_S):
                r0, orows, irows = stripe_geo(s)
                yt = yb[s % 2]
                if s >= 2:
                    vector.wait_ge(sts[s % 2], 32 * (s // 2))
                for q, (cl, ch) in enumerate(((2048, 3072), (3072, OW))):
                    # quarter q covers banks 4+2q, 5+2q
                    vector.wait_ge(pe_sem, NBANKS * s + 2 * q + 6)
                    nc.vector.tensor_scalar_add(
                        out=yt[0:orows, cl:ch],
                        in0=ps[0:orows, cl:ch],
                        scalar1=bv32[0:orows, 0:1],
                    ).then_inc(vec_sem, 1)

    return nc


def _host_prep(input, weight, bias):
    input = np.ascontiguousarray(input, dtype=np.float32)
    weight = np.asarray(weight, dtype=np.float32)
    bias = np.asarray(bias, dtype=np.float32)
    w16 = weight.astype(np.float16)

    mc = np.zeros((128, MC_COLS), dtype=np.float16)
    # band matrices packed side by side: mc[:, dj*126+m] column m of M_dj,
    # M_dj[k, m] = weight[k-m, dj] for 0 <= k-m < KH
    idx = np.arange(STRIPE)
    for dj in range(KW):
        for di in range(KH):
            mc[idx + di, dj * STRIPE + idx] = w16[di, dj]
    # bias column (fp16; |err| <= 2^-11*|b|, well within tolerance)
    mc[:, MC_BIAS] = np.float16(bias[0])
    # stub band: mc[10*dj + m + di, MC_MZ + m] = w[di, dj]
    for dj in range(KW):
        for m in range(STUB_ROWS):
            for di in range(KH):
                mc[10 * dj + m + di, MC_MZ + m] = w16[di, dj]

    x16 = input.astype(np.float16)
    in_maps = []
    for i in range(NCORES):
        r0 = i * CROWS
        sl = x16[r0:r0 + IN_ROWS]
        if sl.shape[0] < IN_ROWS:
            sl = np.concatenate(
                [sl, np.zeros((IN_ROWS - sl.shape[0], W), np.float16)], axis=0
            )
        in_maps.append({"x": np.ascontiguousarray(sl), "mc": mc})
    return in_maps


def _run(input, weight, bias, **spmd_kwargs):
    global _cached
    if _cached is None:
        _cached = _build()
    in_maps = _host_prep(input, weight, bias)
    res = run_bass_kernel_spmd(
        _cached, in_maps, core_ids=list(range(NCORES)), **spmd_kwargs
    )
    out = np.empty((OH, OW), dtype=np.float32)
    for i in range(NCORES):
        r0 = i * CROWS
        rows = min(CROWS, OH - r0)
        out[r0:r0 + rows] = res.results[i]["y"][:rows].astype(np.float32)
    return out, res


def kernel(input, weight, bias):
    out, _ = _run(input, weight, bias)
    return out
